# revision 9
# baseline (speedup 1.0000x reference)
"""CortexIIBlock TRN2 Bass kernel — 8-core data-parallel over (batch, seq-half).

Layout: activations transposed [feature, time] on-chip. All matmuls fp32r
(11-bit mantissa, fp32 accumulate). Depthwise causal convs = shifted
scalar_tensor_tensor FMAs on DVE. RMSNorm partition-reductions via
ones-matmul on PE; per-row scales broadcast via K=1 rank-1 matmuls.

Per core: T = 128 history + 2048 payload rows. History rows are the
previous 128 rows of the same sequence (zeros at sequence start); only
the up-projection "val" half is computed for them (conv lookback <= 6).
"""
import numpy as np

D = 1024
DFF = 4096
B = 4
S = 4096
H = 128          # history rows per shard
R = 2048         # payload rows per shard
T = H + R        # 2176
NCT = D // 128   # 8 channel tiles
NB = 4           # payload blocks of 512
BN = 512
EPS = 1e-6

_CACHE = {}


def _build():
    import concourse.bacc as bacc
    import concourse.mybir as mybir
    import concourse.tile as tile

    F32 = mybir.dt.float32
    F32R = mybir.dt.float32r
    AF = mybir.ActivationFunctionType
    MUL = mybir.AluOpType.mult
    ADD = mybir.AluOpType.add

    nc = bacc.Bacc(None, target_bir_lowering=False)
    _lp = nc.allow_low_precision(reason="fp32r (11-bit mantissa) rounding is intentional")
    _lp.__enter__()

    xT_d = nc.dram_tensor("xT", [D, T], F32R, kind="ExternalInput")
    up_d = nc.dram_tensor("up_sb", [16, 128, D], F32R, kind="ExternalInput")
    down_d = nc.dram_tensor("down_sb", [8, 128, D], F32R, kind="ExternalInput")
    wg_d = nc.dram_tensor("wg_sb", [32, 128, D], F32R, kind="ExternalInput")
    wu_d = nc.dram_tensor("wu_sb", [32, 128, D], F32R, kind="ExternalInput")
    wo_d = nc.dram_tensor("wo_sb", [8, 128, DFF], F32R, kind="ExternalInput")
    sg_d = nc.dram_tensor("sg_sb", [NCT, 128, 3], F32R, kind="ExternalInput")
    ln1_d = nc.dram_tensor("ln1_sb", [NCT, 128, 1], F32, kind="ExternalInput")
    ln2_d = nc.dram_tensor("ln2_sb", [NCT, 128, 1], F32, kind="ExternalInput")
    taps_d = nc.dram_tensor("taps_sb", [NCT, 128, 15], F32, kind="ExternalInput")
    ones128_d = nc.dram_tensor("ones128", [128, 1], F32R, kind="ExternalInput")
    one1_d = nc.dram_tensor("one1", [1, 128], F32R, kind="ExternalInput")
    one11_d = nc.dram_tensor("one11", [1, 1], F32R, kind="ExternalInput")
    yT_d = nc.dram_tensor("yT", [D, R], F32, kind="ExternalOutput")

    with tile.TileContext(nc) as tc:
        with (
            tc.tile_pool(name="const", bufs=1) as cpool,
            tc.tile_pool(name="x2p", bufs=1, space="DRAM") as x2pool,
        ):
            # constants
            ones128 = cpool.tile([128, 1], F32R, tag="c_ones", name="c_ones")
            nc.sync.dma_start(ones128[:], ones128_d[:])
            one1 = cpool.tile([1, 128], F32R, tag="c_one1", name="c_one1")
            nc.sync.dma_start(one1[:], one1_d[:])
            one11 = cpool.tile([1, 1], F32R, tag="c_one11", name="c_one11")
            nc.sync.dma_start(one11[:], one11_d[:])
            eps_t = cpool.tile([1, 1], F32, tag="c_eps", name="c_eps")
            nc.vector.memset(eps_t[:], EPS)
            sg_t = cpool.tile([128, NCT, 3], F32R, tag="c_sg", name="c_sg")
            for c in range(NCT):
                nc.sync.dma_start(sg_t[:, c, :], sg_d[c])
            ln1_t = cpool.tile([128, NCT], F32, tag="c_ln1", name="c_ln1")
            ln2_t = cpool.tile([128, NCT], F32, tag="c_ln2", name="c_ln2")
            for c in range(NCT):
                nc.sync.dma_start(ln1_t[:, c:c + 1], ln1_d[c])
                nc.sync.dma_start(ln2_t[:, c:c + 1], ln2_d[c])
            taps_t = cpool.tile([128, NCT, 15], F32, tag="c_taps", name="c_taps")
            for c in range(NCT):
                nc.sync.dma_start(taps_t[:, c, :], taps_d[c])

            # post-mixer residual stream x2, staged in DRAM
            x2d = x2pool.tile([D, R], F32, tag="x2d", name="x2d")

            # ---------------- mixer ----------------
            prev_val = None
            with (
                tc.tile_pool(name="valp", bufs=2) as vpool,
                tc.tile_pool(name="mx", bufs=2) as mx,
                tc.tile_pool(name="wmix", bufs=3) as wmx,
                tc.tile_pool(name="psA", bufs=2, space="PSUM") as psA,
                tc.tile_pool(name="psB", bufs=2, space="PSUM") as psB,
                tc.tile_pool(name="pssm", bufs=2, space="PSUM") as pssm,
            ):
                for bi in range(NB + 1):
                    hist = bi == 0
                    N = H if hist else BN
                    c0 = 0 if hist else H + (bi - 1) * BN

                    xb = []
                    for c in range(NCT):
                        t_ = mx.tile([128, BN], F32R, tag=f"xb{c}", name=f"xb{c}", bufs=1)
                        nc.sync.dma_start(t_[:, :N], xT_d[c * 128:(c + 1) * 128, c0:c0 + N])
                        xb.append(t_)

                    # --- rmsnorm: msum = sum_d x^2 ---
                    msum = pssm.tile([1, BN], F32, tag="msum", name="msum", bufs=1)
                    for c in range(NCT):
                        sq = mx.tile([128, BN], F32R, tag="sq", name="sq")
                        nc.scalar.activation(sq[:, :N], xb[c][:, :N].bitcast(F32), AF.Square)
                        nc.tensor.matmul(msum[:, :N], ones128[:], sq[:, :N],
                                         start=(c == 0), stop=(c == NCT - 1))
                    sd = mx.tile([1, BN], F32, tag="sd", name="sd")
                    nc.scalar.activation(sd[:, :N], msum[:, :N], AF.Sqrt,
                                         bias=eps_t[:], scale=1.0 / D)
                    rstd = mx.tile([1, BN], F32R, tag="rstd", name="rstd")
                    nc.vector.reciprocal(rstd[:, :N], sd[:, :N])
                    rsb_ps = pssm.tile([128, BN], F32, tag="pbc", name="rsbp", bufs=1)
                    nc.tensor.matmul(rsb_ps[:, :N], one1[:], rstd[:, :N],
                                     start=True, stop=True)
                    rsb = mx.tile([128, BN], F32, tag="rsb", name="rsb")
                    nc.scalar.copy(rsb[:, :N], rsb_ps[:, :N])

                    # h = x * rstd * ln1w   (per c-tile, one fused DVE op)
                    hT = []
                    for c in range(NCT):
                        h_ = mx.tile([128, BN], F32R, tag=f"h{c}", name=f"h{c}", bufs=1)
                        nc.vector.scalar_tensor_tensor(
                            out=h_[:, :N], in0=xb[c][:, :N].bitcast(F32),
                            scalar=ln1_t[:, c:c + 1],
                            in1=rsb[:, :N], op0=MUL, op1=MUL)
                        hT.append(h_)

                    # --- val half of up-projection (m 8..15) ---
                    val = []
                    for c in range(NCT):
                        v_ = vpool.tile([128, 6 + BN], F32, tag=f"val{c}", name=f"val{c}")
                        val.append(v_)
                        if not hist:
                            nc.vector.tensor_copy(v_[:, 0:6], prev_val[c][:, (H if bi == 1 else BN):(H if bi == 1 else BN) + 6])
                    for m in range(NCT):
                        wt = wmx.tile([128, D], F32R, tag="wmix", name="wmix")
                        nc.sync.dma_start(wt[:], up_d[8 + m])
                        pv = psA.tile([128, BN], F32, tag="pmm", name="pval")
                        for k in range(NCT):
                            nc.tensor.matmul(pv[:, :N], wt[:, k * 128:(k + 1) * 128],
                                             hT[k][:, :N], start=(k == 0), stop=(k == NCT - 1))
                        nc.scalar.copy(val[m][:, 6:6 + N], pv[:, :N])

                    if hist:
                        prev_val = val
                        continue

                    # --- scale gates: sg = softmax(h @ sgw.T), per-row j ---
                    ej = []
                    for j in range(3):
                        pj = pssm.tile([1, BN], F32, tag="psg", name="psg", bufs=1)
                        for k in range(NCT):
                            nc.tensor.matmul(pj[:, :N], sg_t[:, k, j:j + 1], hT[k][:, :N],
                                             start=(k == 0), stop=(k == NCT - 1))
                        e_ = mx.tile([1, BN], F32R, tag=f"e{j}", name=f"e{j}")
                        nc.scalar.activation(e_[:, :N], pj[:, :N], AF.Exp)
                        ej.append(e_)
                    es = mx.tile([1, BN], F32, tag="es", name="es")
                    nc.vector.tensor_add(es[:, :N], ej[0][:, :N].bitcast(F32), ej[1][:, :N].bitcast(F32))
                    nc.vector.tensor_add(es[:, :N], es[:, :N], ej[2][:, :N].bitcast(F32))
                    erec = mx.tile([1, BN], F32, tag="erec", name="erec")
                    nc.vector.reciprocal(erec[:, :N], es[:, :N])
                    swb = []
                    for j in range(3):
                        swj = mx.tile([1, BN], F32R, tag="swj", name="swj")
                        nc.vector.tensor_mul(swj[:, :N], ej[j][:, :N].bitcast(F32), erec[:, :N])
                        pb_ = pssm.tile([128, BN], F32, tag="pbc", name="pswb", bufs=1)
                        nc.tensor.matmul(pb_[:, :N], one1[:], swj[:, :N], start=True, stop=True)
                        sb_ = mx.tile([128, BN], F32, tag=f"swb{j}", name=f"swb{j}")
                        nc.scalar.copy(sb_[:, :N], pb_[:, :N])
                        swb.append(sb_)

                    # --- gate (up m 0..7), conv, mix, z ---
                    zT = []
                    for c in range(NCT):
                        wt = wmx.tile([128, D], F32R, tag="wmix", name="wmix")
                        nc.sync.dma_start(wt[:], up_d[c])
                        pg = psA.tile([128, BN], F32, tag="pmm", name="pgate")
                        for k in range(NCT):
                            nc.tensor.matmul(pg[:, :N], wt[:, k * 128:(k + 1) * 128],
                                             hT[k][:, :N], start=(k == 0), stop=(k == NCT - 1))
                        gate = mx.tile([128, BN], F32, tag="gate", name="gate")
                        nc.scalar.activation(gate[:, :N], pg[:, :N], AF.Sigmoid)

                        v_ = val[c]
                        # c_fine (3 taps), c_med (5), c_coarse (7); tap jj order
                        convs = []
                        for (nt, base) in ((3, 0), (5, 3), (7, 8)):
                            ct_ = mx.tile([128, BN], F32, tag=f"cv{len(convs)}", name=f"cv{len(convs)}")
                            nc.vector.tensor_scalar_mul(
                                ct_[:, :N], v_[:, 6:6 + N], taps_t[:, c, base:base + 1])
                            for j in range(1, nt):
                                nc.vector.scalar_tensor_tensor(
                                    out=ct_[:, :N], in0=v_[:, 6 - j:6 - j + N],
                                    scalar=taps_t[:, c, base + j:base + j + 1],
                                    in1=ct_[:, :N], op0=MUL, op1=ADD)
                            convs.append(ct_)
                        acc = mx.tile([128, BN], F32, tag="acc", name="acc")
                        nc.vector.tensor_mul(acc[:, :N], convs[0][:, :N], swb[0][:, :N])
                        for j in (1, 2):
                            u_ = mx.tile([128, BN], F32, tag="mixu", name="mixu")
                            nc.vector.tensor_mul(u_[:, :N], convs[j][:, :N], swb[j][:, :N])
                            nc.vector.tensor_add(acc[:, :N], acc[:, :N], u_[:, :N])
                        z_ = mx.tile([128, BN], F32R, tag=f"z{c}", name=f"z{c}", bufs=1)
                        nc.vector.tensor_mul(z_[:, :N], acc[:, :N], gate[:, :N])
                        zT.append(z_)

                    # --- down projection + residual -> x2 ---
                    for m in range(NCT):
                        wt = wmx.tile([128, D], F32R, tag="wmix", name="wmix")
                        nc.sync.dma_start(wt[:], down_d[m])
                        pm = psB.tile([128, BN], F32, tag="pmix", name="pmix")
                        for k in range(NCT):
                            nc.tensor.matmul(pm[:, :N], wt[:, k * 128:(k + 1) * 128],
                                             zT[k][:, :N], start=(k == 0), stop=(k == NCT - 1))
                        x2b = mx.tile([128, BN], F32, tag="x2b", name="x2b")
                        nc.vector.tensor_add(
                            x2b[:, :N], xb[m][:, :N].bitcast(F32), pm[:, :N])
                        nc.sync.dma_start(
                            x2d[m * 128:(m + 1) * 128, c0 - H:c0 - H + N], x2b[:, :N])

                    prev_val = val

            # ---------------- FFN ----------------
            with (
                tc.tile_pool(name="fx", bufs=2) as fx,
                tc.tile_pool(name="pp", bufs=1) as pp,
                tc.tile_pool(name="wgu", bufs=4) as wgu,
                tc.tile_pool(name="wop", bufs=2) as wop,
                tc.tile_pool(name="psG", bufs=2, space="PSUM") as psG,
                tc.tile_pool(name="psU", bufs=2, space="PSUM") as psU,
                tc.tile_pool(name="psY", bufs=2, space="PSUM") as psY,
                tc.tile_pool(name="pss2", bufs=2, space="PSUM") as pss2,
            ):
                for rb in range(NB):
                    r0 = rb * BN
                    x2b = []
                    for c in range(NCT):
                        t_ = fx.tile([128, BN], F32, tag=f"x2r{c}", name=f"x2r{c}", bufs=1)
                        nc.sync.dma_start(t_[:], x2d[c * 128:(c + 1) * 128, r0:r0 + BN])
                        x2b.append(t_)
                    # rmsnorm(x2)
                    msum = pss2.tile([1, BN], F32, tag="msum2", name="msum2", bufs=1)
                    for c in range(NCT):
                        sq = fx.tile([128, BN], F32R, tag="sq2", name="sq2")
                        nc.scalar.activation(sq[:], x2b[c][:], AF.Square)
                        nc.tensor.matmul(msum[:], ones128[:], sq[:],
                                         start=(c == 0), stop=(c == NCT - 1))
                    sd = fx.tile([1, BN], F32, tag="sd2", name="sd2")
                    nc.scalar.activation(sd[:], msum[:], AF.Sqrt,
                                         bias=eps_t[:], scale=1.0 / D)
                    rstd = fx.tile([1, BN], F32R, tag="rstd2", name="rstd2")
                    nc.vector.reciprocal(rstd[:], sd[:])
                    rsb_ps = pss2.tile([128, BN], F32, tag="rsbp2", name="rsbp2", bufs=1)
                    nc.tensor.matmul(rsb_ps[:], one1[:], rstd[:], start=True, stop=True)
                    rsb = fx.tile([128, BN], F32, tag="rsb2", name="rsb2")
                    nc.scalar.copy(rsb[:], rsb_ps[:])
                    h2 = []
                    for c in range(NCT):
                        h_ = fx.tile([128, BN], F32R, tag=f"h2_{c}", name=f"h2_{c}", bufs=1)
                        nc.vector.scalar_tensor_tensor(
                            out=h_[:], in0=x2b[c][:],
                            scalar=ln2_t[:, c:c + 1],
                            in1=rsb[:], op0=MUL, op1=MUL)
                        h2.append(h_)

                    # g/u + silu + product -> p tiles
                    pT = []
                    for m in range(32):
                        wtg = wgu.tile([128, D], F32R, tag="wg", name="wg")
                        nc.sync.dma_start(wtg[:], wg_d[m])
                        pg = psG.tile([128, BN], F32, tag="pg", name="pg")
                        for k in range(NCT):
                            nc.tensor.matmul(pg[:], wtg[:, k * 128:(k + 1) * 128],
                                             h2[k][:], start=(k == 0), stop=(k == NCT - 1))
                        wtu = wgu.tile([128, D], F32R, tag="wu", name="wu")
                        nc.sync.dma_start(wtu[:], wu_d[m])
                        pu = psU.tile([128, BN], F32, tag="pu", name="pu")
                        for k in range(NCT):
                            nc.tensor.matmul(pu[:], wtu[:, k * 128:(k + 1) * 128],
                                             h2[k][:], start=(k == 0), stop=(k == NCT - 1))
                        tg = fx.tile([128, BN], F32, tag="tg", name="tg")
                        nc.scalar.activation(tg[:], pg[:], AF.Silu)
                        p_ = pp.tile([128, BN], F32R, tag=f"p{m}", name=f"p{m}")
                        nc.vector.tensor_mul(p_[:], tg[:], pu[:])
                        pT.append(p_)

                    # wo projection + residual -> out
                    for m in range(NCT):
                        wa = wop.tile([128, 2048], F32R, tag="woA", name="woA")
                        nc.sync.dma_start(wa[:], wo_d[m][:, 0:2048])
                        wb = wop.tile([128, 2048], F32R, tag="woB", name="woB")
                        nc.sync.dma_start(wb[:], wo_d[m][:, 2048:4096])
                        py = psY.tile([128, BN], F32, tag="py", name="py")
                        for k in range(32):
                            wt = wa if k < 16 else wb
                            ks = (k % 16) * 128
                            nc.tensor.matmul(py[:], wt[:, ks:ks + 128], pT[k][:],
                                             start=(k == 0), stop=(k == 31))
                        yo = fx.tile([128, BN], F32, tag="yo", name="yo")
                        nc.vector.tensor_add(yo[:], x2b[m][:], py[:])
                        nc.sync.dma_start(yT_d[m * 128:(m + 1) * 128, r0:r0 + BN], yo[:])

    if not nc.is_finalized():
        nc.finalize()
    return nc


def _host_prep(x, ln1_w, ln2_w, w_fine, w_medium, w_coarse, sg_w, up_w, down_w, wg, wu, wo):
    f = np.float32
    up_sb = np.ascontiguousarray(
        up_w.T.reshape(NCT, 128, 16, 128).transpose(2, 1, 0, 3).reshape(16, 128, D), f)
    down_sb = np.ascontiguousarray(
        down_w.T.reshape(NCT, 128, 8, 128).transpose(2, 1, 0, 3).reshape(8, 128, D), f)
    wg_sb = np.ascontiguousarray(
        wg.T.reshape(NCT, 128, 32, 128).transpose(2, 1, 0, 3).reshape(32, 128, D), f)
    wu_sb = np.ascontiguousarray(
        wu.T.reshape(NCT, 128, 32, 128).transpose(2, 1, 0, 3).reshape(32, 128, D), f)
    wo_sb = np.ascontiguousarray(
        wo.T.reshape(32, 128, 8, 128).transpose(2, 1, 0, 3).reshape(8, 128, DFF), f)
    sg_sb = np.ascontiguousarray(sg_w.T.reshape(NCT, 128, 3), f)
    ln1_sb = np.ascontiguousarray(ln1_w.reshape(NCT, 128, 1), f)
    ln2_sb = np.ascontiguousarray(ln2_w.reshape(NCT, 128, 1), f)
    taps = np.zeros((NCT, 128, 15), f)
    for (w_, nt, base) in ((w_fine, 3, 0), (w_medium, 5, 3), (w_coarse, 7, 8)):
        for j in range(nt):
            taps[:, :, base + j] = w_[:, 0, nt - 1 - j].reshape(NCT, 128)
    shared = dict(up_sb=up_sb, down_sb=down_sb, wg_sb=wg_sb, wu_sb=wu_sb,
                  wo_sb=wo_sb, sg_sb=sg_sb, ln1_sb=ln1_sb, ln2_sb=ln2_sb,
                  taps_sb=taps,
                  ones128=np.ones((128, 1), f), one1=np.ones((1, 128), f),
                  one11=np.ones((1, 1), f))
    in_maps = []
    for core in range(8):
        b, half = core // 2, core % 2
        if half == 0:
            histx = np.zeros((H, D), f)
            pay = x[b, 0:R]
        else:
            histx = x[b, R - H:R]
            pay = x[b, R:S]
        xT = np.ascontiguousarray(np.concatenate([histx, pay], 0).T, f)
        in_maps.append({**shared, "xT": xT})
    return in_maps


def kernel(**inputs):
    from concourse.bass_utils import run_bass_kernel_spmd
    if "nc" not in _CACHE:
        _CACHE["nc"] = _build()
    nc = _CACHE["nc"]
    in_maps = _host_prep(**{k: np.asarray(v) for k, v in inputs.items()})
    res = run_bass_kernel_spmd(nc, in_maps, core_ids=list(range(8)))
    out = np.empty((B, S, D), np.float32)
    for core in range(8):
        b, half = core // 2, core % 2
        out[b, half * R:(half + 1) * R] = res.results[core]["yT"].T
    return out


# revision 13
# speedup vs baseline: 97.5820x; 97.5820x over previous
"""CortexIIBlock TRN2 Bass kernel — 8-core data-parallel over (batch, seq-half).

Layout: activations transposed [feature, time] on-chip. All matmuls fp32r
(11-bit mantissa, fp32 accumulate). Depthwise causal convs = shifted
scalar_tensor_tensor FMAs on DVE. RMSNorm partition-reductions via
ones-matmul on PE; per-row scales broadcast via K=1 rank-1 matmuls.

Per core: T = 128 history + 2048 payload rows. History rows are the
previous 128 rows of the same sequence (zeros at sequence start); only
the up-projection "val" half is computed for them (conv lookback <= 6).
"""
import numpy as np

D = 1024
DFF = 4096
B = 4
S = 4096
H = 128          # history rows per shard
R = 2048         # payload rows per shard
T = H + R        # 2176
NCT = D // 128   # 8 channel tiles
NB = 4           # payload blocks of 512
BN = 512
EPS = 1e-6

_CACHE = {}


def _build():
    import concourse.bacc as bacc
    import concourse.mybir as mybir
    import concourse.tile as tile

    F32 = mybir.dt.float32
    F32R = mybir.dt.float32r
    BF16 = mybir.dt.bfloat16
    AF = mybir.ActivationFunctionType
    MUL = mybir.AluOpType.mult
    ADD = mybir.AluOpType.add

    nc = bacc.Bacc(None, target_bir_lowering=False)
    _lp = nc.allow_low_precision(reason="fp32r (11-bit mantissa) rounding is intentional")
    _lp.__enter__()

    xT_d = nc.dram_tensor("xT", [D, T], F32R, kind="ExternalInput")
    up_d = nc.dram_tensor("up_sb", [16, 128, D], F32R, kind="ExternalInput")
    down_d = nc.dram_tensor("down_sb", [8, 128, D], F32R, kind="ExternalInput")
    wg_d = nc.dram_tensor("wg_sb", [32, 128, D], F32R, kind="ExternalInput")
    wu_d = nc.dram_tensor("wu_sb", [32, 128, D], F32R, kind="ExternalInput")
    wo_d = nc.dram_tensor("wo_sb", [8, 128, DFF], F32R, kind="ExternalInput")
    sg_d = nc.dram_tensor("sg_sb", [NCT, 128, 3], F32R, kind="ExternalInput")
    ln1_d = nc.dram_tensor("ln1_sb", [NCT, 128, 1], F32, kind="ExternalInput")
    ln2_d = nc.dram_tensor("ln2_sb", [NCT, 128, 1], F32, kind="ExternalInput")
    taps_d = nc.dram_tensor("taps_sb", [NCT, 128, 15], F32, kind="ExternalInput")
    ones128_d = nc.dram_tensor("ones128", [128, 1], F32R, kind="ExternalInput")
    one1_d = nc.dram_tensor("one1", [1, 128], F32R, kind="ExternalInput")
    one11_d = nc.dram_tensor("one11", [1, 1], F32R, kind="ExternalInput")
    yT_d = nc.dram_tensor("yT", [D, R], F32, kind="ExternalOutput")

    with tile.TileContext(nc) as tc:
        with (
            tc.tile_pool(name="const", bufs=1) as cpool,
            tc.tile_pool(name="x2p", bufs=1, space="DRAM") as x2pool,
        ):
            # constants
            ones128 = cpool.tile([128, 1], F32R, tag="c_ones", name="c_ones")
            nc.sync.dma_start(ones128[:], ones128_d[:])
            one1 = cpool.tile([1, 128], F32R, tag="c_one1", name="c_one1")
            nc.sync.dma_start(one1[:], one1_d[:])
            one11 = cpool.tile([1, 1], F32R, tag="c_one11", name="c_one11")
            nc.sync.dma_start(one11[:], one11_d[:])
            eps_t = cpool.tile([1, 1], F32, tag="c_eps", name="c_eps")
            nc.vector.memset(eps_t[:], EPS)
            sg_t = cpool.tile([128, NCT, 3], F32R, tag="c_sg", name="c_sg")
            for c in range(NCT):
                nc.sync.dma_start(sg_t[:, c, :], sg_d[c])
            ln1_t = cpool.tile([128, NCT], F32, tag="c_ln1", name="c_ln1")
            ln2_t = cpool.tile([128, NCT], F32, tag="c_ln2", name="c_ln2")
            for c in range(NCT):
                nc.sync.dma_start(ln1_t[:, c:c + 1], ln1_d[c])
                nc.sync.dma_start(ln2_t[:, c:c + 1], ln2_d[c])
            taps_t = cpool.tile([128, NCT, 15], F32, tag="c_taps", name="c_taps")
            for c in range(NCT):
                nc.sync.dma_start(taps_t[:, c, :], taps_d[c])

            # post-mixer residual stream x2, staged in DRAM
            x2d = x2pool.tile([D, R], F32, tag="x2d", name="x2d")

            # ---------------- mixer ----------------
            prev_val = None
            with (
                tc.tile_pool(name="valp", bufs=2) as vpool,
                tc.tile_pool(name="mx", bufs=2) as mx,
                tc.tile_pool(name="wmix", bufs=3) as wmx,
                tc.tile_pool(name="psA", bufs=2, space="PSUM") as psA,
                tc.tile_pool(name="psB", bufs=2, space="PSUM") as psB,
                tc.tile_pool(name="pssm", bufs=2, space="PSUM") as pssm,
            ):
                for bi in range(NB + 1):
                    hist = bi == 0
                    N = H if hist else BN
                    c0 = 0 if hist else H + (bi - 1) * BN

                    xb = []
                    for c in range(NCT):
                        t_ = mx.tile([128, BN], F32R, tag=f"xb{c}", name=f"xb{c}", bufs=2)
                        nc.sync.dma_start(t_[:, :N], xT_d[c * 128:(c + 1) * 128, c0:c0 + N])
                        xb.append(t_)

                    # --- rmsnorm: msum = sum_d x^2 ---
                    msum = pssm.tile([1, BN], F32, tag="msum", name="msum", bufs=1)
                    for c in range(NCT):
                        sq = mx.tile([128, BN], F32R, tag="sq", name="sq")
                        nc.scalar.activation(sq[:, :N], xb[c][:, :N].bitcast(F32), AF.Square)
                        nc.tensor.matmul(msum[:, :N], ones128[:], sq[:, :N],
                                         start=(c == 0), stop=(c == NCT - 1))
                    sd = mx.tile([1, BN], F32, tag="sd", name="sd")
                    nc.scalar.activation(sd[:, :N], msum[:, :N], AF.Sqrt,
                                         bias=eps_t[:], scale=1.0 / D)
                    rstd = mx.tile([1, BN], F32R, tag="rstd", name="rstd")
                    nc.vector.reciprocal(rstd[:, :N], sd[:, :N])
                    rsb_ps = pssm.tile([128, BN], F32, tag="pbc", name="rsbp", bufs=1)
                    nc.tensor.matmul(rsb_ps[:, :N], one1[:], rstd[:, :N],
                                     start=True, stop=True)
                    rsb = mx.tile([128, BN], F32, tag="rsb", name="rsb")
                    nc.scalar.copy(rsb[:, :N], rsb_ps[:, :N])

                    # h = x * rstd * ln1w   (per c-tile, one fused DVE op)
                    hT = []
                    for c in range(NCT):
                        h_ = mx.tile([128, BN], F32R, tag=f"h{c}", name=f"h{c}", bufs=2)
                        nc.vector.scalar_tensor_tensor(
                            out=h_[:, :N], in0=xb[c][:, :N].bitcast(F32),
                            scalar=ln1_t[:, c:c + 1],
                            in1=rsb[:, :N], op0=MUL, op1=MUL)
                        hT.append(h_)

                    # --- val half of up-projection (m 8..15) ---
                    val = []
                    for c in range(NCT):
                        v_ = vpool.tile([128, 6 + BN], F32, tag=f"val{c}", name=f"val{c}")
                        val.append(v_)
                        if not hist:
                            nc.vector.tensor_copy(v_[:, 0:6], prev_val[c][:, (H if bi == 1 else BN):(H if bi == 1 else BN) + 6])
                    for m in range(NCT):
                        wt = wmx.tile([128, D], F32R, tag="wmix", name="wmix")
                        nc.sync.dma_start(wt[:], up_d[8 + m])
                        pv = psA.tile([128, BN], F32, tag="pmm", name="pval")
                        for k in range(NCT):
                            nc.tensor.matmul(pv[:, :N], wt[:, k * 128:(k + 1) * 128],
                                             hT[k][:, :N], start=(k == 0), stop=(k == NCT - 1))
                        nc.scalar.copy(val[m][:, 6:6 + N], pv[:, :N])

                    if hist:
                        prev_val = val
                        continue

                    # --- scale gates: sg = softmax(h @ sgw.T), per-row j ---
                    ej = []
                    for j in range(3):
                        pj = pssm.tile([1, BN], F32, tag="psg", name="psg", bufs=1)
                        for k in range(NCT):
                            nc.tensor.matmul(pj[:, :N], sg_t[:, k, j:j + 1], hT[k][:, :N],
                                             start=(k == 0), stop=(k == NCT - 1))
                        e_ = mx.tile([1, BN], F32R, tag=f"e{j}", name=f"e{j}")
                        nc.scalar.activation(e_[:, :N], pj[:, :N], AF.Exp)
                        ej.append(e_)
                    es = mx.tile([1, BN], F32, tag="es", name="es")
                    nc.vector.tensor_add(es[:, :N], ej[0][:, :N].bitcast(F32), ej[1][:, :N].bitcast(F32))
                    nc.vector.tensor_add(es[:, :N], es[:, :N], ej[2][:, :N].bitcast(F32))
                    erec = mx.tile([1, BN], F32, tag="erec", name="erec")
                    nc.vector.reciprocal(erec[:, :N], es[:, :N])
                    swb = []
                    for j in range(3):
                        swj = mx.tile([1, BN], F32R, tag="swj", name="swj")
                        nc.vector.tensor_mul(swj[:, :N], ej[j][:, :N].bitcast(F32), erec[:, :N])
                        pb_ = pssm.tile([128, BN], F32, tag="pbc", name="pswb", bufs=1)
                        nc.tensor.matmul(pb_[:, :N], one1[:], swj[:, :N], start=True, stop=True)
                        sb_ = mx.tile([128, BN], F32, tag=f"swb{j}", name=f"swb{j}")
                        nc.scalar.copy(sb_[:, :N], pb_[:, :N])
                        swb.append(sb_)

                    # --- gate (up m 0..7), conv, mix, z ---
                    zT = []
                    for c in range(NCT):
                        wt = wmx.tile([128, D], F32R, tag="wmix", name="wmix")
                        nc.sync.dma_start(wt[:], up_d[c])
                        pg = psA.tile([128, BN], F32, tag="pmm", name="pgate")
                        for k in range(NCT):
                            nc.tensor.matmul(pg[:, :N], wt[:, k * 128:(k + 1) * 128],
                                             hT[k][:, :N], start=(k == 0), stop=(k == NCT - 1))
                        gate = mx.tile([128, BN], F32, tag="gate", name="gate")
                        nc.scalar.activation(gate[:, :N], pg[:, :N], AF.Sigmoid)

                        v_ = val[c]
                        # c_fine (3 taps), c_med (5), c_coarse (7); tap jj order
                        convs = []
                        for (nt, base) in ((3, 0), (5, 3), (7, 8)):
                            ct_ = mx.tile([128, BN], F32, tag=f"cv{len(convs)}", name=f"cv{len(convs)}")
                            nc.vector.tensor_scalar_mul(
                                ct_[:, :N], v_[:, 6:6 + N], taps_t[:, c, base:base + 1])
                            for j in range(1, nt):
                                nc.vector.scalar_tensor_tensor(
                                    out=ct_[:, :N], in0=v_[:, 6 - j:6 - j + N],
                                    scalar=taps_t[:, c, base + j:base + j + 1],
                                    in1=ct_[:, :N], op0=MUL, op1=ADD)
                            convs.append(ct_)
                        acc = mx.tile([128, BN], F32, tag="acc", name="acc")
                        nc.vector.tensor_mul(acc[:, :N], convs[0][:, :N], swb[0][:, :N])
                        for j in (1, 2):
                            u_ = mx.tile([128, BN], F32, tag="mixu", name="mixu")
                            nc.vector.tensor_mul(u_[:, :N], convs[j][:, :N], swb[j][:, :N])
                            nc.vector.tensor_add(acc[:, :N], acc[:, :N], u_[:, :N])
                        z_ = mx.tile([128, BN], F32R, tag=f"z{c}", name=f"z{c}", bufs=1)
                        nc.vector.tensor_mul(z_[:, :N], acc[:, :N], gate[:, :N])
                        zT.append(z_)

                    # --- down projection + residual -> x2 ---
                    for m in range(NCT):
                        wt = wmx.tile([128, D], F32R, tag="wmix", name="wmix")
                        nc.sync.dma_start(wt[:], down_d[m])
                        pm = psB.tile([128, BN], F32, tag="pmix", name="pmix")
                        for k in range(NCT):
                            nc.tensor.matmul(pm[:, :N], wt[:, k * 128:(k + 1) * 128],
                                             zT[k][:, :N], start=(k == 0), stop=(k == NCT - 1))
                        x2b = mx.tile([128, BN], F32, tag="x2b", name="x2b")
                        nc.vector.tensor_add(
                            x2b[:, :N], xb[m][:, :N].bitcast(F32), pm[:, :N])
                        nc.sync.dma_start(
                            x2d[m * 128:(m + 1) * 128, c0 - H:c0 - H + N], x2b[:, :N])

                    prev_val = val

            # ---------------- FFN ----------------
            with (
                tc.tile_pool(name="fx", bufs=2) as fx,
                tc.tile_pool(name="pp", bufs=1) as pp,
                tc.tile_pool(name="wgu", bufs=6) as wgu,
                tc.tile_pool(name="wop", bufs=2) as wop,
                tc.tile_pool(name="psG", bufs=2, space="PSUM") as psG,
                tc.tile_pool(name="psU", bufs=2, space="PSUM") as psU,
                tc.tile_pool(name="psY", bufs=2, space="PSUM") as psY,
                tc.tile_pool(name="pss2", bufs=2, space="PSUM") as pss2,
            ):
                for rb in range(NB):
                    r0 = rb * BN
                    x2b = []
                    for c in range(NCT):
                        t_ = fx.tile([128, BN], F32, tag=f"x2r{c}", name=f"x2r{c}", bufs=1)
                        nc.sync.dma_start(t_[:], x2d[c * 128:(c + 1) * 128, r0:r0 + BN])
                        x2b.append(t_)
                    # rmsnorm(x2)
                    msum = pss2.tile([1, BN], F32, tag="msum2", name="msum2", bufs=1)
                    for c in range(NCT):
                        sq = fx.tile([128, BN], F32R, tag="sq2", name="sq2")
                        nc.scalar.activation(sq[:], x2b[c][:], AF.Square)
                        nc.tensor.matmul(msum[:], ones128[:], sq[:],
                                         start=(c == 0), stop=(c == NCT - 1))
                    sd = fx.tile([1, BN], F32, tag="sd2", name="sd2")
                    nc.scalar.activation(sd[:], msum[:], AF.Sqrt,
                                         bias=eps_t[:], scale=1.0 / D)
                    rstd = fx.tile([1, BN], F32R, tag="rstd2", name="rstd2")
                    nc.vector.reciprocal(rstd[:], sd[:])
                    rsb_ps = pss2.tile([128, BN], F32, tag="rsbp2", name="rsbp2", bufs=1)
                    nc.tensor.matmul(rsb_ps[:], one1[:], rstd[:], start=True, stop=True)
                    rsb = fx.tile([128, BN], F32, tag="rsb2", name="rsb2")
                    nc.scalar.copy(rsb[:], rsb_ps[:])
                    h2 = []
                    for c in range(NCT):
                        h_ = fx.tile([128, BN], F32R, tag=f"h2_{c}", name=f"h2_{c}", bufs=1)
                        nc.vector.scalar_tensor_tensor(
                            out=h_[:], in0=x2b[c][:],
                            scalar=ln2_t[:, c:c + 1],
                            in1=rsb[:], op0=MUL, op1=MUL)
                        h2.append(h_)

                    # g/u + silu + product -> p tiles
                    pT = []
                    for m in range(32):
                        wtg = wgu.tile([128, D], F32R, tag="wg", name="wg")
                        nc.sync.dma_start(wtg[:], wg_d[m])
                        pg = psG.tile([128, BN], F32, tag="pg", name="pg")
                        for k in range(NCT):
                            nc.tensor.matmul(pg[:], wtg[:, k * 128:(k + 1) * 128],
                                             h2[k][:], start=(k == 0), stop=(k == NCT - 1))
                        wtu = wgu.tile([128, D], F32R, tag="wu", name="wu")
                        nc.sync.dma_start(wtu[:], wu_d[m])
                        pu = psU.tile([128, BN], F32, tag="pu", name="pu")
                        for k in range(NCT):
                            nc.tensor.matmul(pu[:], wtu[:, k * 128:(k + 1) * 128],
                                             h2[k][:], start=(k == 0), stop=(k == NCT - 1))
                        tg = fx.tile([128, BN], F32, tag="tg", name="tg")
                        nc.scalar.activation(tg[:], pg[:], AF.Silu)
                        p_ = pp.tile([128, BN], F32R, tag=f"p{m}", name=f"p{m}")
                        nc.vector.tensor_mul(p_[:], tg[:], pu[:])
                        pT.append(p_)

                    # wo projection + residual -> out
                    for m in range(NCT):
                        wa = wop.tile([128, 2048], F32R, tag="woA", name="woA")
                        nc.sync.dma_start(wa[:], wo_d[m][:, 0:2048])
                        wb = wop.tile([128, 2048], F32R, tag="woB", name="woB")
                        nc.sync.dma_start(wb[:], wo_d[m][:, 2048:4096])
                        py = psY.tile([128, BN], F32, tag="py", name="py")
                        for k in range(32):
                            wt = wa if k < 16 else wb
                            ks = (k % 16) * 128
                            nc.tensor.matmul(py[:], wt[:, ks:ks + 128], pT[k][:],
                                             start=(k == 0), stop=(k == 31))
                        yo = fx.tile([128, BN], F32, tag="yo", name="yo")
                        nc.vector.tensor_add(yo[:], x2b[m][:], py[:])
                        nc.sync.dma_start(yT_d[m * 128:(m + 1) * 128, r0:r0 + BN], yo[:])

    if not nc.is_finalized():
        nc.finalize()
    return nc


def _host_prep(x, ln1_w, ln2_w, w_fine, w_medium, w_coarse, sg_w, up_w, down_w, wg, wu, wo):
    f = np.float32
    up_sb = np.ascontiguousarray(
        up_w.T.reshape(NCT, 128, 16, 128).transpose(2, 1, 0, 3).reshape(16, 128, D), f)
    down_sb = np.ascontiguousarray(
        down_w.T.reshape(NCT, 128, 8, 128).transpose(2, 1, 0, 3).reshape(8, 128, D), f)
    wg_sb = np.ascontiguousarray(
        wg.T.reshape(NCT, 128, 32, 128).transpose(2, 1, 0, 3).reshape(32, 128, D), f)
    wu_sb = np.ascontiguousarray(
        wu.T.reshape(NCT, 128, 32, 128).transpose(2, 1, 0, 3).reshape(32, 128, D), f)
    wo_sb = np.ascontiguousarray(
        wo.T.reshape(32, 128, 8, 128).transpose(2, 1, 0, 3).reshape(8, 128, DFF), f)
    sg_sb = np.ascontiguousarray(sg_w.T.reshape(NCT, 128, 3), f)
    ln1_sb = np.ascontiguousarray(ln1_w.reshape(NCT, 128, 1), f)
    ln2_sb = np.ascontiguousarray(ln2_w.reshape(NCT, 128, 1), f)
    taps = np.zeros((NCT, 128, 15), f)
    for (w_, nt, base) in ((w_fine, 3, 0), (w_medium, 5, 3), (w_coarse, 7, 8)):
        for j in range(nt):
            taps[:, :, base + j] = w_[:, 0, nt - 1 - j].reshape(NCT, 128)
    shared = dict(up_sb=up_sb, down_sb=down_sb, wg_sb=wg_sb, wu_sb=wu_sb,
                  wo_sb=wo_sb, sg_sb=sg_sb, ln1_sb=ln1_sb, ln2_sb=ln2_sb,
                  taps_sb=taps,
                  ones128=np.ones((128, 1), f), one1=np.ones((1, 128), f),
                  one11=np.ones((1, 1), f))
    in_maps = []
    for core in range(8):
        b, half = core // 2, core % 2
        if half == 0:
            histx = np.zeros((H, D), f)
            pay = x[b, 0:R]
        else:
            histx = x[b, R - H:R]
            pay = x[b, R:S]
        xT = np.ascontiguousarray(np.concatenate([histx, pay], 0).T, f)
        in_maps.append({**shared, "xT": xT})
    return in_maps


def kernel(**inputs):
    from concourse.bass_utils import run_bass_kernel_spmd
    if "nc" not in _CACHE:
        _CACHE["nc"] = _build()
    nc = _CACHE["nc"]
    in_maps = _host_prep(**{k: np.asarray(v) for k, v in inputs.items()})
    res = run_bass_kernel_spmd(nc, in_maps, core_ids=list(range(8)))
    out = np.empty((B, S, D), np.float32)
    for core in range(8):
        b, half = core // 2, core % 2
        out[b, half * R:(half + 1) * R] = res.results[core]["yT"].T
    return out
